# revision 11
# baseline (speedup 1.0000x reference)
"""MoE inverted-dispatch expert bank kernel for 8x Trainium2 NeuronCores.

Strategy (expert parallelism + load packing + SBUF-native DRAM layouts):
  - Host: replicate the reference routing (stable argsort -> per-expert rank,
    capacity drop), build a transposed per-expert token buffer, cast to bf16.
  - Experts are sorted by token count and dealt round-robin into 8 "slots" x
    8 cores: every core runs the same per-slot capacity schedule (SPMD, one
    program) while matmuls only cover the tokens actually routed.
  - All DRAM tensors are pre-tiled on the host into SBUF-native [128, X]
    layout so every DMA moves large contiguous runs per partition; weight
    halves alternate between the two HWDGE rings (sync/scalar).
  - Device (per core): per expert slot, grouped 2-layer MLP with weights
    stationary, token buffer moving in [feature, token] layout:
    hT[f, C] = gelu(W1-tiles.T @ bufT), yT[d, C] = W2-tiles @ hT.
    Expert pipeline is skewed (L1(e+1) runs before L2(e)) so ScalarE GELU
    latency never blocks the Tensor engine; GELU + output copies are batched
    two m-tiles per PSUM bank to amortize per-instruction overhead.
  - Host: gather yT, scatter-combine to [N, k, d], loads = counts / N.
"""

import os
import numpy as np
import ml_dtypes

N_TOK = 4096
K_ACT = 2
D_MODEL = 512
D_FF = 2048
N_EXP = 64
CAP = 256
N_CORES = 8
E_PER = N_EXP // N_CORES

P = 128
KT1 = D_MODEL // P   # 4  k-tiles layer 1
MT1 = D_FF // P      # 16 m-tiles layer 1
KT2 = D_FF // P      # 16 k-tiles layer 2
MT2 = D_MODEL // P   # 4  m-tiles layer 2
W1W = KT1 * D_FF     # 8192 cols in sbuf-native w1 layout
W2W = KT2 * D_MODEL  # 8192 cols in sbuf-native w2 layout

LAST_EXEC_TIME_NS = None

_CACHE = {}


def _install_trace_hook():
    try:
        from antenv.axon_hooks import get_axon_ntff_profile_hook  # noqa: F401
        return True
    except ImportError:
        pass
    try:
        import sys, types
        import trn_agent_boot.trn_boot as tb
        hook = tb._ntff_profile_via_ctypes('/opt/axon/libaxon_pjrt.so')
        mod = types.ModuleType("antenv.axon_hooks")
        mod.get_axon_ntff_profile_hook = lambda: hook
        mod.set_axon_ntff_profile_hook = lambda h: None
        sys.modules['antenv.axon_hooks'] = mod
        import antenv
        antenv.axon_hooks = mod
        return True
    except Exception:
        return False


def _to_sbuf_layout(a, kt):
    """[kt*128, X] -> [128, kt*X] partition-major tiling (single expert)."""
    rows, x = a.shape
    assert rows == kt * P
    return a.reshape(kt, P, x).transpose(1, 0, 2).reshape(P, kt * x)


def _build_kernel(caps):
    """caps: per-slot moving widths (tokens per expert slot), len E_PER."""
    import concourse.tile as tile
    import concourse.mybir as mybir
    from concourse import bacc
    from concourse.bass import ds

    bf16 = mybir.dt.bfloat16
    f32 = mybir.dt.float32
    H1 = W1W // 2
    H2 = W2W // 2
    xoff = np.concatenate([[0], np.cumsum([KT1 * c for c in caps])])
    yoff = np.concatenate([[0], np.cumsum([MT2 * c for c in caps])])

    nc = bacc.Bacc("TRN2", target_bir_lowering=False, debug=False)
    bufT = nc.dram_tensor("bufT", [P, int(xoff[-1])], bf16, kind="ExternalInput")
    w1 = nc.dram_tensor("w1", [E_PER, P, W1W], bf16, kind="ExternalInput")
    w2 = nc.dram_tensor("w2", [E_PER, P, W2W], bf16, kind="ExternalInput")
    yT = nc.dram_tensor("yT", [P, int(yoff[-1])], bf16, kind="ExternalOutput")

    with tile.TileContext(nc) as tc:
        with (
            tc.tile_pool(name="xpool", bufs=3) as xpool,
            tc.tile_pool(name="w1pool", bufs=8) as w1pool,
            tc.tile_pool(name="w2pool", bufs=8) as w2pool,
            tc.tile_pool(name="hpool", bufs=3) as hpool,
            tc.tile_pool(name="ypool", bufs=3) as ypool,
            tc.tile_pool(name="ps1", bufs=6, space="PSUM") as ps1pool,
            tc.tile_pool(name="ps2", bufs=2, space="PSUM") as ps2pool,
        ):
            warm = xpool.tile([P, 16], bf16, tag="warm")
            nc.vector.memset(warm[:], 0.0)
            nc.scalar.activation(
                warm[:], warm[:], mybir.ActivationFunctionType.Gelu
            )

            def layer1(e):
                C = caps[e]
                # w1 dram layout is m-major: col = m*512 + k*128 + c.
                # Half tiles give PE per-half dependencies: m 0-7 start
                # after only 1MB has landed.
                w1a = w1pool.tile([P, H1], bf16, tag="w1")
                w1b = w1pool.tile([P, H1], bf16, tag="w1")
                if e == 0:
                    # critical startup path: spread across both rings
                    nc.sync.dma_start(w1a[:, :H1 // 2], w1[e, :, :H1 // 2])
                    nc.scalar.dma_start(
                        w1a[:, H1 // 2:], w1[e, :, ds(H1 // 2, H1 // 2)]
                    )
                    nc.sync.dma_start(
                        w1b[:, :H1 // 2], w1[e, :, ds(H1, H1 // 2)]
                    )
                    nc.scalar.dma_start(
                        w1b[:, H1 // 2:], w1[e, :, ds(H1 + H1 // 2, H1 // 2)]
                    )
                else:
                    nc.sync.dma_start(w1a[:], w1[e, :, :H1])
                    nc.sync.dma_start(w1b[:], w1[e, :, H1:])
                xt = xpool.tile([P, KT1 * C], bf16, tag="x")
                nc.gpsimd.dma_start(xt[:], bufT[:, ds(int(xoff[e]), KT1 * C)])
                # w2 dram layout is k-major: col = k*512 + m*128 + c.
                w2a = w2pool.tile([P, H2], bf16, tag="w2")
                w2b = w2pool.tile([P, H2], bf16, tag="w2")
                nc.scalar.dma_start(w2a[:], w2[e, :, :H2])
                nc.scalar.dma_start(w2b[:], w2[e, :, H2:])

                ht = hpool.tile([P, KT2, C], bf16, tag="h")
                for g in range(MT1 // 2):
                    ps = ps1pool.tile([P, 2, C], f32, tag="ps1")
                    for half in range(2):
                        m = 2 * g + half
                        for k in range(KT1):
                            wsrc = w1a if m < MT1 // 2 else w1b
                            nc.tensor.matmul(
                                ps[:, half, :],
                                lhsT=wsrc[:, ds((m % (MT1 // 2)) * (KT1 * P) + k * P, P)],
                                rhs=xt[:, ds(k * C, C)],
                                start=(half == 0 and k == 0),
                                stop=(half == 1 and k == KT1 - 1),
                            )
                    nc.scalar.activation(
                        ht[:, 2 * g:2 * g + 2, :], ps[:],
                        mybir.ActivationFunctionType.Gelu,
                    )
                return ht, w2a, w2b

            def layer2(e, ht, w2a, w2b):
                C = caps[e]
                yt = ypool.tile([P, MT2 * C], bf16, tag="y")
                for g in range(MT2 // 2):
                    ps = ps2pool.tile([P, 2, C], f32, tag="ps2")
                    for half in range(2):
                        m = 2 * g + half
                        for k in range(KT2):
                            wsrc = w2a if k < KT2 // 2 else w2b
                            nc.tensor.matmul(
                                ps[:, half, :],
                                lhsT=wsrc[:, ds((k % (KT2 // 2)) * D_MODEL + m * P, P)],
                                rhs=ht[:, k, :],
                                start=(half == 0 and k == 0),
                                stop=(half == 1 and k == KT2 - 1),
                            )
                    nc.vector.tensor_copy(yt[:, ds(2 * g * C, 2 * C)], ps[:])
                nc.scalar.dma_start(yT[:, ds(int(yoff[e]), MT2 * C)], yt[:])

            # skewed pipeline: L1(e+1) before L2(e)
            prev = None
            for e in range(E_PER):
                state = layer1(e)
                if prev is not None:
                    layer2(e - 1, *prev)
                prev = state
            layer2(E_PER - 1, *prev)

    nc.compile()
    return nc


def kernel(hidden_states, selected_experts, expert_masks, W1, W2):
    global LAST_EXEC_TIME_NS
    from concourse.bass_utils import run_bass_kernel_spmd

    hidden = np.ascontiguousarray(np.asarray(hidden_states, dtype=np.float32))
    sel = np.asarray(selected_experts).astype(np.int64)
    W1 = np.asarray(W1, dtype=np.float32)
    W2 = np.asarray(W2, dtype=np.float32)

    # ---- host dispatch (mirrors reference routing exactly) ----
    flat_e = sel.reshape(-1)
    S = flat_e.shape[0]
    order = np.argsort(flat_e, kind="stable")
    e_sorted = flat_e[order]
    counts = np.bincount(flat_e, minlength=N_EXP)
    offsets = np.cumsum(counts) - counts
    rank = np.arange(S, dtype=np.int64) - offsets[e_sorted]
    tok = order // K_ACT
    slot = order % K_ACT
    valid = rank < CAP
    ev, rv, tv, sv = e_sorted[valid], rank[valid], tok[valid], slot[valid]

    bf = ml_dtypes.bfloat16
    hidden16 = hidden.astype(bf)
    bufT = np.zeros((N_EXP, D_MODEL, CAP), dtype=bf)
    bufT[ev, :, rv] = hidden16[tv]
    W1_16 = W1.astype(bf)
    W2_16 = W2.astype(bf)

    # ---- pack: sort experts by load, deal bands of 8 across the 8 cores ----
    counts_used = np.minimum(counts, CAP)
    perm = np.argsort(-counts_used, kind="stable")
    perm_js = perm.reshape(E_PER, N_CORES)  # [slot j, core i] -> expert id
    caps = tuple(
        int(np.ceil(max(int(counts_used[perm_js[j]].max()), 16) / 16) * 16)
        for j in range(E_PER)
    )

    trace = os.environ.get("KERNEL_TRACE", "0") == "1" and _install_trace_hook()

    if caps not in _CACHE:
        _CACHE[caps] = _build_kernel(caps)
    nc = _CACHE[caps]

    in_maps = []
    for i in range(N_CORES):
        sl = perm_js[:, i]
        # bufT packed: per slot j, [P, KT1*C_j] segments concatenated
        xsegs = []
        for j in range(E_PER):
            e = sl[j]
            c = caps[j]
            seg = bufT[e].reshape(KT1, P, CAP)[:, :, :c]
            xsegs.append(seg.transpose(1, 0, 2).reshape(P, KT1 * c))
        in_maps.append(
            {
                "bufT": np.ascontiguousarray(np.concatenate(xsegs, axis=1)),
                "w1": np.ascontiguousarray(
                    np.stack([
                        W1_16[e].reshape(KT1, P, MT1, P)
                        .transpose(1, 2, 0, 3).reshape(P, W1W)
                        for e in sl
                    ])
                ),
                "w2": np.ascontiguousarray(
                    np.stack([_to_sbuf_layout(W2_16[e], KT2) for e in sl])
                ),
            }
        )

    res = run_bass_kernel_spmd(
        nc, in_maps, core_ids=list(range(N_CORES)), trace=trace
    )
    LAST_EXEC_TIME_NS = res.exec_time_ns

    yoff = np.concatenate([[0], np.cumsum([MT2 * c for c in caps])]).astype(int)
    yT = np.zeros((N_EXP, D_MODEL, CAP), dtype=np.float32)
    for i in range(N_CORES):
        out = res.results[i]["yT"]  # [P, sum MT2*C_j] bf16
        for j in range(E_PER):
            e = perm_js[j, i]
            c = caps[j]
            seg = out[:, yoff[j]:yoff[j + 1]].astype(np.float32)
            yT[e, :, :c] = seg.reshape(P, MT2, c).transpose(1, 0, 2).reshape(
                D_MODEL, c
            )

    # ---- host combine ----
    gathered = yT[ev, :, rv]
    expert_outputs = np.zeros((N_TOK, K_ACT, D_MODEL), dtype=np.float32)
    expert_outputs[tv, sv] = gathered
    expert_loads = counts.astype(np.float32) / N_TOK
    return expert_outputs, expert_loads


# revision 12
# speedup vs baseline: 1.0055x; 1.0055x over previous
"""MoE inverted-dispatch expert bank kernel for 8x Trainium2 NeuronCores.

Strategy (expert parallelism + load packing + SBUF-native DRAM layouts):
  - Host: replicate the reference routing (stable argsort -> per-expert rank,
    capacity drop), build a transposed per-expert token buffer, cast to bf16.
  - Experts are sorted by token count and dealt round-robin into 8 "slots" x
    8 cores: every core runs the same per-slot capacity schedule (SPMD, one
    program) while matmuls only cover the tokens actually routed.
  - All DRAM tensors are pre-tiled on the host into SBUF-native [128, X]
    layout so every DMA moves large contiguous runs per partition; weight
    halves alternate between the two HWDGE rings (sync/scalar).
  - Device (per core): per expert slot, grouped 2-layer MLP with weights
    stationary, token buffer moving in [feature, token] layout:
    hT[f, C] = gelu(W1-tiles.T @ bufT), yT[d, C] = W2-tiles @ hT.
    Expert pipeline is skewed (L1(e+1) runs before L2(e)) so ScalarE GELU
    latency never blocks the Tensor engine; GELU + output copies are batched
    two m-tiles per PSUM bank to amortize per-instruction overhead.
  - Host: gather yT, scatter-combine to [N, k, d], loads = counts / N.
"""

import os
import numpy as np
import ml_dtypes

N_TOK = 4096
K_ACT = 2
D_MODEL = 512
D_FF = 2048
N_EXP = 64
CAP = 256
N_CORES = 8
E_PER = N_EXP // N_CORES

P = 128
KT1 = D_MODEL // P   # 4  k-tiles layer 1
MT1 = D_FF // P      # 16 m-tiles layer 1
KT2 = D_FF // P      # 16 k-tiles layer 2
MT2 = D_MODEL // P   # 4  m-tiles layer 2
W1W = KT1 * D_FF     # 8192 cols in sbuf-native w1 layout
W2W = KT2 * D_MODEL  # 8192 cols in sbuf-native w2 layout

LAST_EXEC_TIME_NS = None

_CACHE = {}


def _install_trace_hook():
    try:
        from antenv.axon_hooks import get_axon_ntff_profile_hook  # noqa: F401
        return True
    except ImportError:
        pass
    try:
        import sys, types
        import trn_agent_boot.trn_boot as tb
        hook = tb._ntff_profile_via_ctypes('/opt/axon/libaxon_pjrt.so')
        mod = types.ModuleType("antenv.axon_hooks")
        mod.get_axon_ntff_profile_hook = lambda: hook
        mod.set_axon_ntff_profile_hook = lambda h: None
        sys.modules['antenv.axon_hooks'] = mod
        import antenv
        antenv.axon_hooks = mod
        return True
    except Exception:
        return False


def _to_sbuf_layout(a, kt):
    """[kt*128, X] -> [128, kt*X] partition-major tiling (single expert)."""
    rows, x = a.shape
    assert rows == kt * P
    return a.reshape(kt, P, x).transpose(1, 0, 2).reshape(P, kt * x)


def _build_kernel(caps):
    """caps: per-slot moving widths (tokens per expert slot), len E_PER."""
    import concourse.tile as tile
    import concourse.mybir as mybir
    from concourse import bacc
    from concourse.bass import ds

    bf16 = mybir.dt.bfloat16
    f32 = mybir.dt.float32
    H1 = W1W // 2
    H2 = W2W // 2
    xoff = np.concatenate([[0], np.cumsum([KT1 * c for c in caps])])
    yoff = np.concatenate([[0], np.cumsum([MT2 * c for c in caps])])

    nc = bacc.Bacc("TRN2", target_bir_lowering=False, debug=False)
    bufT = nc.dram_tensor("bufT", [P, int(xoff[-1])], bf16, kind="ExternalInput")
    w1 = nc.dram_tensor("w1", [E_PER, P, W1W], bf16, kind="ExternalInput")
    w2 = nc.dram_tensor("w2", [E_PER, P, W2W], bf16, kind="ExternalInput")
    yT = nc.dram_tensor("yT", [P, int(yoff[-1])], bf16, kind="ExternalOutput")

    with tile.TileContext(nc) as tc:
        with (
            tc.tile_pool(name="xpool", bufs=3) as xpool,
            tc.tile_pool(name="w1pool", bufs=8) as w1pool,
            tc.tile_pool(name="w2pool", bufs=8) as w2pool,
            tc.tile_pool(name="hpool", bufs=4) as hpool,
            tc.tile_pool(name="ypool", bufs=3) as ypool,
            tc.tile_pool(name="ps1", bufs=6, space="PSUM") as ps1pool,
            tc.tile_pool(name="ps2", bufs=2, space="PSUM") as ps2pool,
        ):
            def layer1(e):
                C = caps[e]
                # w1 dram layout is m-major: col = m*512 + k*128 + c.
                # Half tiles give PE per-half dependencies: m 0-7 start
                # after only 1MB has landed.
                w1a = w1pool.tile([P, H1], bf16, tag="w1")
                w1b = w1pool.tile([P, H1], bf16, tag="w1")
                if e == 0:
                    # critical startup path: spread across both rings
                    nc.sync.dma_start(w1a[:, :H1 // 2], w1[e, :, :H1 // 2])
                    nc.scalar.dma_start(
                        w1a[:, H1 // 2:], w1[e, :, ds(H1 // 2, H1 // 2)]
                    )
                    nc.sync.dma_start(
                        w1b[:, :H1 // 2], w1[e, :, ds(H1, H1 // 2)]
                    )
                    nc.scalar.dma_start(
                        w1b[:, H1 // 2:], w1[e, :, ds(H1 + H1 // 2, H1 // 2)]
                    )
                else:
                    nc.sync.dma_start(w1a[:], w1[e, :, :H1])
                    nc.sync.dma_start(w1b[:], w1[e, :, H1:])
                xt = xpool.tile([P, KT1 * C], bf16, tag="x")
                nc.gpsimd.dma_start(xt[:], bufT[:, ds(int(xoff[e]), KT1 * C)])
                # w2 dram layout is k-major: col = k*512 + m*128 + c.
                w2a = w2pool.tile([P, H2], bf16, tag="w2")
                w2b = w2pool.tile([P, H2], bf16, tag="w2")
                nc.scalar.dma_start(w2a[:], w2[e, :, :H2])
                nc.scalar.dma_start(w2b[:], w2[e, :, H2:])

                if e == 0:
                    warm = xpool.tile([P, 16], bf16, tag="warm")
                    nc.vector.memset(warm[:], 0.0)
                    nc.scalar.activation(
                        warm[:], warm[:], mybir.ActivationFunctionType.Gelu
                    )

                ht = hpool.tile([P, KT2, C], bf16, tag="h")
                for g in range(MT1 // 2):
                    ps = ps1pool.tile([P, 2, C], f32, tag="ps1")
                    for half in range(2):
                        m = 2 * g + half
                        for k in range(KT1):
                            wsrc = w1a if m < MT1 // 2 else w1b
                            nc.tensor.matmul(
                                ps[:, half, :],
                                lhsT=wsrc[:, ds((m % (MT1 // 2)) * (KT1 * P) + k * P, P)],
                                rhs=xt[:, ds(k * C, C)],
                                start=(half == 0 and k == 0),
                                stop=(half == 1 and k == KT1 - 1),
                            )
                    nc.scalar.activation(
                        ht[:, 2 * g:2 * g + 2, :], ps[:],
                        mybir.ActivationFunctionType.Gelu,
                    )
                return ht, w2a, w2b

            def layer2(e, ht, w2a, w2b):
                C = caps[e]
                yt = ypool.tile([P, MT2 * C], bf16, tag="y")
                for g in range(MT2 // 2):
                    ps = ps2pool.tile([P, 2, C], f32, tag="ps2")
                    for half in range(2):
                        m = 2 * g + half
                        for k in range(KT2):
                            wsrc = w2a if k < KT2 // 2 else w2b
                            nc.tensor.matmul(
                                ps[:, half, :],
                                lhsT=wsrc[:, ds((k % (KT2 // 2)) * D_MODEL + m * P, P)],
                                rhs=ht[:, k, :],
                                start=(half == 0 and k == 0),
                                stop=(half == 1 and k == KT2 - 1),
                            )
                    nc.vector.tensor_copy(yt[:, ds(2 * g * C, 2 * C)], ps[:])
                nc.scalar.dma_start(yT[:, ds(int(yoff[e]), MT2 * C)], yt[:])

            # skewed pipeline: L1(e+1) before L2(e)
            prev = None
            for e in range(E_PER):
                state = layer1(e)
                if prev is not None:
                    layer2(e - 1, *prev)
                prev = state
            layer2(E_PER - 1, *prev)

    nc.compile()
    return nc


def kernel(hidden_states, selected_experts, expert_masks, W1, W2):
    global LAST_EXEC_TIME_NS
    from concourse.bass_utils import run_bass_kernel_spmd

    hidden = np.ascontiguousarray(np.asarray(hidden_states, dtype=np.float32))
    sel = np.asarray(selected_experts).astype(np.int64)
    W1 = np.asarray(W1, dtype=np.float32)
    W2 = np.asarray(W2, dtype=np.float32)

    # ---- host dispatch (mirrors reference routing exactly) ----
    flat_e = sel.reshape(-1)
    S = flat_e.shape[0]
    order = np.argsort(flat_e, kind="stable")
    e_sorted = flat_e[order]
    counts = np.bincount(flat_e, minlength=N_EXP)
    offsets = np.cumsum(counts) - counts
    rank = np.arange(S, dtype=np.int64) - offsets[e_sorted]
    tok = order // K_ACT
    slot = order % K_ACT
    valid = rank < CAP
    ev, rv, tv, sv = e_sorted[valid], rank[valid], tok[valid], slot[valid]

    bf = ml_dtypes.bfloat16
    hidden16 = hidden.astype(bf)
    bufT = np.zeros((N_EXP, D_MODEL, CAP), dtype=bf)
    bufT[ev, :, rv] = hidden16[tv]
    W1_16 = W1.astype(bf)
    W2_16 = W2.astype(bf)

    # ---- pack: sort experts by load, deal bands of 8 across the 8 cores ----
    counts_used = np.minimum(counts, CAP)
    perm = np.argsort(-counts_used, kind="stable")
    perm_js = perm.reshape(E_PER, N_CORES)  # [slot j, core i] -> expert id
    caps = tuple(
        int(np.ceil(max(int(counts_used[perm_js[j]].max()), 16) / 16) * 16)
        for j in range(E_PER)
    )

    trace = os.environ.get("KERNEL_TRACE", "0") == "1" and _install_trace_hook()

    if caps not in _CACHE:
        _CACHE[caps] = _build_kernel(caps)
    nc = _CACHE[caps]

    in_maps = []
    for i in range(N_CORES):
        sl = perm_js[:, i]
        # bufT packed: per slot j, [P, KT1*C_j] segments concatenated
        xsegs = []
        for j in range(E_PER):
            e = sl[j]
            c = caps[j]
            seg = bufT[e].reshape(KT1, P, CAP)[:, :, :c]
            xsegs.append(seg.transpose(1, 0, 2).reshape(P, KT1 * c))
        in_maps.append(
            {
                "bufT": np.ascontiguousarray(np.concatenate(xsegs, axis=1)),
                "w1": np.ascontiguousarray(
                    np.stack([
                        W1_16[e].reshape(KT1, P, MT1, P)
                        .transpose(1, 2, 0, 3).reshape(P, W1W)
                        for e in sl
                    ])
                ),
                "w2": np.ascontiguousarray(
                    np.stack([_to_sbuf_layout(W2_16[e], KT2) for e in sl])
                ),
            }
        )

    res = run_bass_kernel_spmd(
        nc, in_maps, core_ids=list(range(N_CORES)), trace=trace
    )
    LAST_EXEC_TIME_NS = res.exec_time_ns

    yoff = np.concatenate([[0], np.cumsum([MT2 * c for c in caps])]).astype(int)
    yT = np.zeros((N_EXP, D_MODEL, CAP), dtype=np.float32)
    for i in range(N_CORES):
        out = res.results[i]["yT"]  # [P, sum MT2*C_j] bf16
        for j in range(E_PER):
            e = perm_js[j, i]
            c = caps[j]
            seg = out[:, yoff[j]:yoff[j + 1]].astype(np.float32)
            yT[e, :, :c] = seg.reshape(P, MT2, c).transpose(1, 0, 2).reshape(
                D_MODEL, c
            )

    # ---- host combine ----
    gathered = yT[ev, :, rv]
    expert_outputs = np.zeros((N_TOK, K_ACT, D_MODEL), dtype=np.float32)
    expert_outputs[tv, sv] = gathered
    expert_loads = counts.astype(np.float32) / N_TOK
    return expert_outputs, expert_loads
